# revision 1
# baseline (speedup 1.0000x reference)
"""BoxAttention TRN2 kernel — 8-core data-parallel over the window dim.

Per core: 256 windows x 64 tokens x 384 dim, 12 heads, head_dim 32.
Pipeline per 128-token pair-tile (2 windows), all layouts chosen so no
gather and no cross-core communication is needed:

  xT  (c,tok)  <- DMA-transpose (bf16) or PE-transpose (f32) of x
  qT,kT (kout,tok) <- W_qk^T stationary matmuls, rhs = xT
  v   (tok,kv) <- xT-slice stationary matmuls, rhs = W_v^T  (+ ones col)
  S^T (nk,nq)  <- per (window,head) matmuls, tile_position packed
  E^T          <- exp(S^T) * exp(bias)^T            (bias folded via exp)
  AV  (nq, h*33+d) <- stationary E^T, moving v_ext; col 32 = softmax denom
  attn (tok,c) <- AV * 1/denom
  out (tok,o)  <- attn^T stationary, rhs = W_p^T, + proj_b
"""

import os
import sys
import numpy as np

for _p in ("/opt/trn_rl_repo", "/opt/pypackages"):
    if _p not in sys.path and os.path.isdir(_p):
        sys.path.append(_p)

import ml_dtypes  # noqa: E402

DIM, BOX, H = 384, 4, 12
N = BOX ** 3            # 64 tokens per window
HD = DIM // H           # 32
SCALE = HD ** -0.5
B_ = 2048
NCORES = 8
B_PER = B_ // NCORES    # 256 windows per core
TOK = B_PER * N         # 16384 tokens per core
SUPER = 512             # tokens per super-tile (8 windows)
PAIR = 128              # tokens per pair-tile (2 windows)

MODE = os.environ.get("BOXATTN_MODE", "f32")  # "f32" | "bf16"

_cache = {}


def _build(mode, tok_per_core, reps=1):
    import concourse.bass as bass
    import concourse.mybir as mybir
    import concourse.tile as tile
    from concourse import bacc

    f32 = mybir.dt.float32
    dt = mybir.dt.bfloat16 if mode == "bf16" else f32

    nc = bacc.Bacc("TRN2", target_bir_lowering=False, debug=False)

    x_d = nc.dram_tensor("x", [tok_per_core, DIM], dt, kind="ExternalInput").ap()
    wqk_d = nc.dram_tensor("wqkT", [DIM, 768], dt, kind="ExternalInput").ap()
    wv_d = nc.dram_tensor("wvT", [DIM, DIM], dt, kind="ExternalInput").ap()
    wp_d = nc.dram_tensor("wpT", [DIM, DIM], dt, kind="ExternalInput").ap()
    eb_d = nc.dram_tensor("ebT", [PAIR, 1536], dt, kind="ExternalInput").ap()
    pb_d = nc.dram_tensor("pb", [PAIR, DIM], f32, kind="ExternalInput").ap()
    id_d = nc.dram_tensor("ident", [PAIR, PAIR], f32, kind="ExternalInput").ap()
    out_d = nc.dram_tensor("out", [tok_per_core, DIM], f32, kind="ExternalOutput").ap()

    n_super = tok_per_core // SUPER

    with tile.TileContext(nc) as tc:
        with (
            tc.tile_pool(name="consts", bufs=1) as consts,
            tc.tile_pool(name="xn", bufs=3) as xn_pool,
            tc.tile_pool(name="xt", bufs=3) as xt_pool,
            tc.tile_pool(name="qk", bufs=3) as qk_pool,
            tc.tile_pool(name="v", bufs=3) as v_pool,
            tc.tile_pool(name="er", bufs=4) as er_pool,
            tc.tile_pool(name="et", bufs=4) as et_pool,
            tc.tile_pool(name="av", bufs=4) as av_pool,
            tc.tile_pool(name="avt", bufs=4) as avt_pool,
            tc.tile_pool(name="osb", bufs=4) as o_pool,
            tc.tile_pool(name="inv", bufs=4) as inv_pool,
            tc.tile_pool(name="psA", bufs=2, space="PSUM") as psA,
            tc.tile_pool(name="psS", bufs=4, space="PSUM") as psS,
            tc.tile_pool(name="psB", bufs=2, space="PSUM") as psB,
        ):
            wqk = consts.tile([128, 3, 768], dt)
            nc.sync.dma_start(wqk[:], wqk_d.rearrange("(a p) k -> p a k", p=128))
            wv = consts.tile([128, 3, DIM], dt)
            nc.sync.dma_start(wv[:], wv_d.rearrange("(a p) k -> p a k", p=128))
            wp = consts.tile([128, 3, DIM], dt)
            nc.sync.dma_start(wp[:], wp_d.rearrange("(a p) k -> p a k", p=128))
            eb = consts.tile([PAIR, 1536], dt)
            nc.sync.dma_start(eb[:], eb_d)
            pb = consts.tile([PAIR, DIM], f32)
            nc.sync.dma_start(pb[:], pb_d)
            ident = None
            if mode != "bf16":
                ident = consts.tile([PAIR, PAIR], f32)
                nc.sync.dma_start(ident[:], id_d)
            vbufs = []
            for _i in range(3):
                vper = consts.tile([128, H, 33], dt, tag=f"vper{_i}")
                nc.vector.memset(vper[:, :, 32:33], 1.0)
                vbufs.append(vper)

            for sp in range(n_super * reps):
                t0 = (sp % n_super) * SUPER
                # ---- xT [c, tok] for this super-tile ----
                xt = xt_pool.tile([128, 3, SUPER], dt, tag="xt")
                if mode == "bf16":
                    for cc in range(3):
                        nc.sync.dma_start(
                            out=xt[:, cc, :],
                            in_=x_d[t0 : t0 + SUPER, cc * 128 : (cc + 1) * 128],
                            transpose=True,
                        )
                else:
                    xn = xn_pool.tile([128, 4, DIM], f32, tag="xn")
                    nc.sync.dma_start(
                        xn[:], x_d[t0 : t0 + SUPER, :].rearrange("(b p) c -> p b c", p=128)
                    )
                    for cc in range(3):
                        for tb in range(4):
                            tp = psB.tile([128, 128], f32, tag="bp")
                            nc.tensor.transpose(
                                tp[:], xn[:, tb, cc * 128 : (cc + 1) * 128], ident[:]
                            )
                            nc.scalar.copy(xt[:, cc, tb * 128 : (tb + 1) * 128], tp[:])

                # ---- q,k projections (transposed layout) ----
                qkt = qk_pool.tile([128, 6, SUPER], dt, tag="qkt")
                for j in range(6):
                    ps = psA.tile([128, SUPER], f32, tag="psA")
                    for cc in range(3):
                        nc.tensor.matmul(
                            ps[:],
                            lhsT=wqk[:, cc, j * 128 : (j + 1) * 128],
                            rhs=xt[:, cc, :],
                            start=(cc == 0),
                            stop=(cc == 2),
                        )
                    nc.scalar.copy(qkt[:, j, :], ps[:])

                for blk in range(4):
                    tok0 = t0 + blk * PAIR
                    # ---- v (natural layout, interleaved with ones col) ----
                    vps = psA.tile([128, DIM], f32, tag="psA")
                    for cc in range(3):
                        nc.tensor.matmul(
                            vps[:],
                            lhsT=xt[:, cc, blk * 128 : (blk + 1) * 128],
                            rhs=wv[:, cc, :],
                            start=(cc == 0),
                            stop=(cc == 2),
                        )
                    vsb = vbufs[(sp * 4 + blk) % 3]
                    nc.vector.tensor_copy(
                        vsb[:, :, 0:32], vps[:].rearrange("p (h d) -> p h d", d=32)
                    )

                    # ---- S^T per head: one [32,128]x[32,128] matmul over the
                    # whole pair-tile. Cross-window blocks are garbage; the
                    # bias multiply (eb = 0 there) zeroes them, which makes
                    # E^T block-diagonal so AV is one matmul per head too.
                    # One PSUM bank per PE row-group g=h%4 (concurrent
                    # tile_position matmuls must not share a bank).
                    # Bank g must hold exactly the heads of PE row-group g:
                    # concurrent tile_position matmuls from different row
                    # groups must not write the same PSUM bank.
                    sts = []
                    for _g in range(4):
                        st_g = psS.tile([128, 384], f32, tag="s")
                        sts.append(st_g)
                    for h in range(H):
                        g, j = h % 4, h // 4
                        rp = g * 32
                        f0 = blk * 128
                        nc.tensor.matmul(
                            sts[g][:, j * 128 : (j + 1) * 128],
                            lhsT=qkt[rp : rp + 32, 3 + j, f0 : f0 + 128],
                            rhs=qkt[rp : rp + 32, j, f0 : f0 + 128],
                            start=True,
                            stop=True,
                            tile_position=(rp, 0),
                        )
                    er = er_pool.tile([128, 1536], dt, tag="er")
                    for g in range(4):
                        nc.scalar.activation(
                            er[:, g * 384 : (g + 1) * 384],
                            sts[g][:],
                            mybir.ActivationFunctionType.Exp,
                        )
                    et = et_pool.tile([128, 1536], dt, tag="et")
                    nc.vector.tensor_mul(et[:], er[:], eb[:])

                    # ---- AV (+ denominator in col 32 of each head block) ----
                    avp_t = psB.tile([128, 512], f32, tag="bp")
                    avp = avp_t[:, 0 : H * 33].rearrange("p (h d) -> p h d", d=33)
                    for h in range(H):
                        ec = (h % 4) * 384 + (h // 4) * 128
                        nc.tensor.matmul(
                            avp[:, h, :],
                            lhsT=et[:, ec : ec + 128],
                            rhs=vsb[:, h, :],
                            start=True,
                            stop=True,
                        )
                    inv = inv_pool.tile([128, H], f32, tag="inv")
                    nc.vector.reciprocal(inv[:], avp[:, :, 32])
                    avsb = av_pool.tile([128, H, 32], dt, tag="av")
                    nc.vector.tensor_mul(
                        avsb[:],
                        avp[:, :, 0:32],
                        inv[:, :, None].broadcast_to([128, H, 32]),
                    )

                    # ---- attn^T for the output projection ----
                    avt = avt_pool.tile([128, 3, 128], dt, tag="avt")
                    if mode == "bf16":
                        nc.sync.dma_start(
                            out=avt[:],
                            in_=avsb[:].rearrange("p h d -> p (h d)"),
                            transpose=True,
                        )
                    else:
                        for cc in range(3):
                            tp = psB.tile([128, 128], f32, tag="bp")
                            nc.tensor.transpose(
                                tp[:],
                                avsb[:].rearrange("p h d -> p (h d)")[
                                    :, cc * 128 : (cc + 1) * 128
                                ],
                                ident[:],
                            )
                            nc.scalar.copy(avt[:, cc, :], tp[:])

                    # ---- output projection + bias ----
                    ops = psA.tile([128, DIM], f32, tag="psA")
                    for cc in range(3):
                        nc.tensor.matmul(
                            ops[:],
                            lhsT=avt[:, cc, :],
                            rhs=wp[:, cc, :],
                            start=(cc == 0),
                            stop=(cc == 2),
                        )
                    osb = o_pool.tile([128, DIM], f32, tag="osb")
                    nc.vector.tensor_add(osb[:], ops[:], pb[:])
                    nc.sync.dma_start(out_d[tok0 : tok0 + PAIR, :], osb[:])
    nc.compile()
    return nc


def _get_nc(mode, tok_per_core, reps=1):
    key = (mode, tok_per_core, reps)
    if key not in _cache:
        _cache[key] = _build(mode, tok_per_core, reps)
    return _cache[key]


def _host_prep(x, qkv_w, proj_w, proj_b, bias_table, rel_idx, mode, n_cores):
    np_dt = ml_dtypes.bfloat16 if mode == "bf16" else np.float32
    x = np.asarray(x, np.float32)
    qkv_w = np.asarray(qkv_w, np.float32)
    proj_w = np.asarray(proj_w, np.float32)
    proj_b = np.asarray(proj_b, np.float32)
    bias_table = np.asarray(bias_table, np.float32)
    rel_idx = np.asarray(rel_idx)

    wq = qkv_w[0:DIM] * SCALE
    wk = qkv_w[DIM : 2 * DIM]
    wv = qkv_w[2 * DIM :]
    wqkT = np.concatenate([wq, wk], 0).T.copy().astype(np_dt)  # [384, 768]
    wvT = wv.T.copy().astype(np_dt)
    wpT = proj_w.T.copy().astype(np_dt)

    bias = bias_table[rel_idx.reshape(-1)].reshape(N, N, H)  # [nq, nk, h]
    eb1 = np.exp(bias).transpose(1, 2, 0)  # [nk, h, nq]
    ebT = np.zeros((PAIR, H * PAIR), np.float32)  # cross-window blocks stay 0
    for h in range(H):
        ec = (h % 4) * 384 + (h // 4) * 128
        for w in range(2):
            ebT[w * N : (w + 1) * N, ec + w * N : ec + (w + 1) * N] = eb1[:, h, :]
    ebT = ebT.astype(np_dt)  # [128, 1536]
    pb = np.broadcast_to(proj_b, (PAIR, DIM)).copy().astype(np.float32)
    ident = np.eye(PAIR, dtype=np.float32)

    B = x.shape[0]
    bper = B // n_cores
    xs = x.reshape(B * N, DIM).astype(np_dt)
    in_maps = []
    for c in range(n_cores):
        in_maps.append(
            {
                "x": xs[c * bper * N : (c + 1) * bper * N],
                "wqkT": wqkT,
                "wvT": wvT,
                "wpT": wpT,
                "ebT": ebT,
                "pb": pb,
                "ident": ident,
            }
        )
    return in_maps


def kernel(x, qkv_w, proj_w, proj_b, bias_table, rel_idx):
    from concourse.bass_utils import run_bass_kernel_spmd

    x = np.asarray(x)
    B = x.shape[0]
    n_cores = NCORES
    tok_per_core = (B // n_cores) * N
    nc = _get_nc(MODE, tok_per_core)
    in_maps = _host_prep(x, qkv_w, proj_w, proj_b, bias_table, rel_idx, MODE, n_cores)
    res = run_bass_kernel_spmd(nc, in_maps, list(range(n_cores)))
    out = np.concatenate([r["out"] for r in res.results], 0)
    return out.reshape(B, N, DIM).astype(np.float32)



# revision 2
# speedup vs baseline: 20.1335x; 20.1335x over previous
"""BoxAttention TRN2 kernel — 8-core data-parallel over the window dim.

Per core: 256 windows x 64 tokens x 384 dim, 12 heads, head_dim 32.
Pipeline per 128-token pair-tile (2 windows), all layouts chosen so no
gather and no cross-core communication is needed:

  xT  (c,tok)  <- DMA-transpose (bf16) or PE-transpose (f32) of x
  qT,kT (kout,tok) <- W_qk^T stationary matmuls, rhs = xT
  v   (tok,kv) <- xT-slice stationary matmuls, rhs = W_v^T  (+ ones col)
  S^T (nk,nq)  <- per (window,head) matmuls, tile_position packed
  E^T          <- exp(S^T) * exp(bias)^T            (bias folded via exp)
  AV  (nq, h*33+d) <- stationary E^T, moving v_ext; col 32 = softmax denom
  attn (tok,c) <- AV * 1/denom
  out (tok,o)  <- attn^T stationary, rhs = W_p^T, + proj_b

The super-tile loop is a HARDWARE loop (tc.For_i) — module size is
constant in the iteration count, so the NEFF stays small and the
reps-delta timing in test.py measures pure device execution.
"""

import os
import sys
import numpy as np

for _p in ("/opt/trn_rl_repo", "/opt/pypackages"):
    if _p not in sys.path and os.path.isdir(_p):
        sys.path.append(_p)

import ml_dtypes  # noqa: E402

DIM, BOX, H = 384, 4, 12
N = BOX ** 3            # 64 tokens per window
HD = DIM // H           # 32
SCALE = HD ** -0.5
B_ = 2048
NCORES = 8
B_PER = B_ // NCORES    # 256 windows per core
TOK = B_PER * N         # 16384 tokens per core
SUPER = 512             # tokens per super-tile (8 windows)
PAIR = 128              # tokens per pair-tile (2 windows)
UNROLL = 2              # super-tiles per hardware-loop iteration

MODE = os.environ.get("BOXATTN_MODE", "bf16")  # "f32" | "bf16"

_cache = {}


def _build(mode, tok_per_core, reps=1):
    import concourse.bass as bass
    import concourse.mybir as mybir
    import concourse.tile as tile
    from concourse import bacc

    f32 = mybir.dt.float32
    dt = mybir.dt.bfloat16 if mode == "bf16" else f32

    nc = bacc.Bacc("TRN2", target_bir_lowering=False, debug=False)

    x_d = nc.dram_tensor("x", [tok_per_core, DIM], dt, kind="ExternalInput").ap()
    wqk_d = nc.dram_tensor("wqkT", [DIM, 768], dt, kind="ExternalInput").ap()
    wv_d = nc.dram_tensor("wvT", [DIM, DIM], dt, kind="ExternalInput").ap()
    wp_d = nc.dram_tensor("wpT", [DIM, DIM], dt, kind="ExternalInput").ap()
    eb_d = nc.dram_tensor("ebT", [PAIR, 1536], dt, kind="ExternalInput").ap()
    pb_d = nc.dram_tensor("pb", [PAIR, DIM], f32, kind="ExternalInput").ap()
    id_d = nc.dram_tensor("ident", [PAIR, PAIR], f32, kind="ExternalInput").ap()
    out_d = nc.dram_tensor("out", [tok_per_core, DIM], f32, kind="ExternalOutput").ap()

    n_super = tok_per_core // SUPER
    unroll = UNROLL if n_super % UNROLL == 0 else 1
    n_iter = n_super // unroll

    with tile.TileContext(nc) as tc:
        with (
            tc.tile_pool(name="consts", bufs=1) as consts,
            tc.tile_pool(name="xn", bufs=3) as xn_pool,
            tc.tile_pool(name="xt", bufs=3) as xt_pool,
            tc.tile_pool(name="qk", bufs=2) as qk_pool,
            tc.tile_pool(name="v", bufs=3) as v_pool,
            tc.tile_pool(name="er", bufs=3) as er_pool,
            tc.tile_pool(name="et", bufs=3) as et_pool,
            tc.tile_pool(name="av", bufs=3) as av_pool,
            tc.tile_pool(name="avt", bufs=3) as avt_pool,
            tc.tile_pool(name="osb", bufs=3) as o_pool,
            tc.tile_pool(name="inv", bufs=3) as inv_pool,
            tc.tile_pool(name="psA", bufs=2, space="PSUM") as psA,
            tc.tile_pool(name="psS", bufs=4, space="PSUM") as psS,
            tc.tile_pool(name="psB", bufs=2, space="PSUM") as psB,
        ):
            wqk = consts.tile([128, 3, 768], dt)
            nc.sync.dma_start(wqk[:], wqk_d.rearrange("(a p) k -> p a k", p=128))
            wv = consts.tile([128, 3, DIM], dt)
            nc.sync.dma_start(wv[:], wv_d.rearrange("(a p) k -> p a k", p=128))
            wp = consts.tile([128, 3, DIM], dt)
            nc.sync.dma_start(wp[:], wp_d.rearrange("(a p) k -> p a k", p=128))
            eb = consts.tile([PAIR, 1536], dt)
            nc.sync.dma_start(eb[:], eb_d)
            pb = consts.tile([PAIR, DIM], f32)
            nc.sync.dma_start(pb[:], pb_d)
            ident = None
            if mode != "bf16":
                ident = consts.tile([PAIR, PAIR], f32)
                nc.sync.dma_start(ident[:], id_d)

            def super_body(t0):
                # ---- xT [c, tok] for this super-tile ----
                xt = xt_pool.tile([128, 3, SUPER], dt, tag="xt")
                if mode == "bf16":
                    for cc in range(3):
                        nc.sync.dma_start(
                            out=xt[:, cc, :],
                            in_=x_d[bass.ds(t0, SUPER), cc * 128 : (cc + 1) * 128],
                            transpose=True,
                        )
                else:
                    xn = xn_pool.tile([128, 4, DIM], f32, tag="xn")
                    nc.sync.dma_start(
                        xn[:],
                        x_d[bass.ds(t0, SUPER), :].rearrange("(b p) c -> p b c", p=128),
                    )
                    for cc in range(3):
                        for tb in range(4):
                            tp = psB.tile([128, 128], f32, tag="bp")
                            nc.tensor.transpose(
                                tp[:], xn[:, tb, cc * 128 : (cc + 1) * 128], ident[:]
                            )
                            nc.scalar.copy(xt[:, cc, tb * 128 : (tb + 1) * 128], tp[:])

                # ---- q,k projections (transposed layout) ----
                qkt = qk_pool.tile([128, 6, SUPER], dt, tag="qkt")
                for j in range(6):
                    ps = psA.tile([128, SUPER], f32, tag="psA")
                    for cc in range(3):
                        nc.tensor.matmul(
                            ps[:],
                            lhsT=wqk[:, cc, j * 128 : (j + 1) * 128],
                            rhs=xt[:, cc, :],
                            start=(cc == 0),
                            stop=(cc == 2),
                        )
                    # split psum->sbuf copies between ACT and DVE
                    if j % 2 == 0:
                        nc.scalar.copy(qkt[:, j, :], ps[:])
                    else:
                        nc.vector.tensor_copy(qkt[:, j, :], ps[:])

                for blk in range(4):
                    # ---- v (natural layout, interleaved with ones col) ----
                    vps = psA.tile([128, DIM], f32, tag="psA")
                    for cc in range(3):
                        nc.tensor.matmul(
                            vps[:],
                            lhsT=xt[:, cc, blk * 128 : (blk + 1) * 128],
                            rhs=wv[:, cc, :],
                            start=(cc == 0),
                            stop=(cc == 2),
                        )
                    vsb = v_pool.tile([128, H, 33], dt, tag="v")
                    nc.vector.memset(vsb[:, :, 32:33], 1.0)
                    nc.vector.tensor_copy(
                        vsb[:, :, 0:32], vps[:].rearrange("p (h d) -> p h d", d=32)
                    )

                    # ---- S^T per head: one [32,128]x[32,128] matmul over the
                    # whole pair-tile. Cross-window blocks are garbage; the
                    # bias multiply (eb = 0 there) zeroes them, which makes
                    # E^T block-diagonal so AV is one matmul per head too.
                    # One PSUM bank per PE row-group g=h%4 (concurrent
                    # tile_position matmuls must not share a bank).
                    sts = []
                    for _g in range(4):
                        st_g = psS.tile([128, 384], f32, tag="s")
                        sts.append(st_g)
                    for h in range(H):
                        g, j = h % 4, h // 4
                        rp = g * 32
                        f0 = blk * 128
                        nc.tensor.matmul(
                            sts[g][:, j * 128 : (j + 1) * 128],
                            lhsT=qkt[rp : rp + 32, 3 + j, f0 : f0 + 128],
                            rhs=qkt[rp : rp + 32, j, f0 : f0 + 128],
                            start=True,
                            stop=True,
                            tile_position=(rp, 0),
                        )
                    er = er_pool.tile([128, 1536], dt, tag="er")
                    for g in range(4):
                        nc.scalar.activation(
                            er[:, g * 384 : (g + 1) * 384],
                            sts[g][:],
                            mybir.ActivationFunctionType.Exp,
                        )
                    # bias multiply split between DVE and Pool/GpSimd
                    et = et_pool.tile([128, 1536], dt, tag="et")
                    nc.vector.tensor_mul(et[:, 0:768], er[:, 0:768], eb[:, 0:768])
                    nc.gpsimd.tensor_mul(et[:, 768:1536], er[:, 768:1536], eb[:, 768:1536])

                    # ---- AV (+ denominator in col 32 of each head block) ----
                    avp_t = psB.tile([128, 512], f32, tag="bp")
                    avp = avp_t[:, 0 : H * 33].rearrange("p (h d) -> p h d", d=33)
                    for h in range(H):
                        ec = (h % 4) * 384 + (h // 4) * 128
                        nc.tensor.matmul(
                            avp[:, h, :],
                            lhsT=et[:, ec : ec + 128],
                            rhs=vsb[:, h, :],
                            start=True,
                            stop=True,
                        )
                    inv = inv_pool.tile([128, H], f32, tag="inv")
                    nc.vector.reciprocal(inv[:], avp[:, :, 32])
                    avsb = av_pool.tile([128, H, 32], dt, tag="av")
                    nc.vector.tensor_mul(
                        avsb[:],
                        avp[:, :, 0:32],
                        inv[:, :, None].broadcast_to([128, H, 32]),
                    )

                    # ---- attn^T for the output projection ----
                    avt = avt_pool.tile([128, 3, 128], dt, tag="avt")
                    if mode == "bf16":
                        nc.sync.dma_start(
                            out=avt[:],
                            in_=avsb[:].rearrange("p h d -> p (h d)"),
                            transpose=True,
                        )
                    else:
                        for cc in range(3):
                            tp = psB.tile([128, 128], f32, tag="bp")
                            nc.tensor.transpose(
                                tp[:],
                                avsb[:].rearrange("p h d -> p (h d)")[
                                    :, cc * 128 : (cc + 1) * 128
                                ],
                                ident[:],
                            )
                            nc.scalar.copy(avt[:, cc, :], tp[:])

                    # ---- output projection + bias ----
                    ops = psA.tile([128, DIM], f32, tag="psA")
                    for cc in range(3):
                        nc.tensor.matmul(
                            ops[:],
                            lhsT=avt[:, cc, :],
                            rhs=wp[:, cc, :],
                            start=(cc == 0),
                            stop=(cc == 2),
                        )
                    osb = o_pool.tile([128, DIM], f32, tag="osb")
                    nc.vector.tensor_add(osb[:], ops[:], pb[:])
                    nc.sync.dma_start(
                        out_d[bass.ds(t0 + blk * PAIR, PAIR), :], osb[:]
                    )

            with tc.For_i(0, reps) as _rep:
                with tc.For_i(
                    0, n_iter, hint_engines=(mybir.EngineType.PE,)
                ) as sp:
                    base = sp * (SUPER * unroll)
                    for u in range(unroll):
                        super_body(base + u * SUPER)
    nc.compile()
    return nc


def _get_nc(mode, tok_per_core, reps=1):
    key = (mode, tok_per_core, reps)
    if key not in _cache:
        _cache[key] = _build(mode, tok_per_core, reps)
    return _cache[key]


def _host_prep(x, qkv_w, proj_w, proj_b, bias_table, rel_idx, mode, n_cores):
    np_dt = ml_dtypes.bfloat16 if mode == "bf16" else np.float32
    x = np.asarray(x, np.float32)
    qkv_w = np.asarray(qkv_w, np.float32)
    proj_w = np.asarray(proj_w, np.float32)
    proj_b = np.asarray(proj_b, np.float32)
    bias_table = np.asarray(bias_table, np.float32)
    rel_idx = np.asarray(rel_idx)

    wq = qkv_w[0:DIM] * SCALE
    wk = qkv_w[DIM : 2 * DIM]
    wv = qkv_w[2 * DIM :]
    wqkT = np.concatenate([wq, wk], 0).T.copy().astype(np_dt)  # [384, 768]
    wvT = wv.T.copy().astype(np_dt)
    wpT = proj_w.T.copy().astype(np_dt)

    bias = bias_table[rel_idx.reshape(-1)].reshape(N, N, H)  # [nq, nk, h]
    eb1 = np.exp(bias).transpose(1, 2, 0)  # [nk, h, nq]
    ebT = np.zeros((PAIR, H * PAIR), np.float32)  # cross-window blocks stay 0
    for h in range(H):
        ec = (h % 4) * 384 + (h // 4) * 128
        for w in range(2):
            ebT[w * N : (w + 1) * N, ec + w * N : ec + (w + 1) * N] = eb1[:, h, :]
    ebT = ebT.astype(np_dt)  # [128, 1536]
    pb = np.broadcast_to(proj_b, (PAIR, DIM)).copy().astype(np.float32)
    ident = np.eye(PAIR, dtype=np.float32)

    B = x.shape[0]
    bper = B // n_cores
    xs = x.reshape(B * N, DIM).astype(np_dt)
    in_maps = []
    for c in range(n_cores):
        in_maps.append(
            {
                "x": xs[c * bper * N : (c + 1) * bper * N],
                "wqkT": wqkT,
                "wvT": wvT,
                "wpT": wpT,
                "ebT": ebT,
                "pb": pb,
                "ident": ident,
            }
        )
    return in_maps


def kernel(x, qkv_w, proj_w, proj_b, bias_table, rel_idx):
    from concourse.bass_utils import run_bass_kernel_spmd

    x = np.asarray(x)
    B = x.shape[0]
    n_cores = NCORES
    tok_per_core = (B // n_cores) * N
    nc = _get_nc(MODE, tok_per_core)
    in_maps = _host_prep(x, qkv_w, proj_w, proj_b, bias_table, rel_idx, MODE, n_cores)
    res = run_bass_kernel_spmd(nc, in_maps, list(range(n_cores)))
    out = np.concatenate([r["out"] for r in res.results], 0)
    return out.reshape(B, N, DIM).astype(np.float32)


# revision 37
# speedup vs baseline: 1025.1576x; 50.9180x over previous
"""BoxAttention TRN2 kernel — 8-core data-parallel over the window dim.

Per core: 256 windows x 64 tokens x 384 dim, 12 heads, head_dim 32.
Pipeline per 128-token pair-tile (2 windows), all layouts chosen so no
gather and no cross-core communication is needed:

  xT  (c,tok)  <- DMA-transpose (bf16) or PE-transpose (f32) of x
  qT,kT (kout,tok) <- W_qk^T stationary matmuls, rhs = xT
  v   (tok,kv) <- xT-slice stationary matmuls, rhs = W_v^T  (+ ones col)
  S^T (nk,nq)  <- per (window,head) matmuls, tile_position packed
  E^T          <- exp(S^T) * exp(bias)^T            (bias folded via exp)
  AV  (nq, h*33+d) <- stationary E^T, moving v_ext; col 32 = softmax denom
  attn (tok,c) <- AV * 1/denom
  out (tok,o)  <- attn^T stationary, rhs = W_p^T, + proj_b

The super-tile loop is a HARDWARE loop (tc.For_i) — module size is
constant in the iteration count, so the NEFF stays small and the
reps-delta timing in test.py measures pure device execution.
"""

import os
import sys
import numpy as np

for _p in ("/opt/trn_rl_repo", "/opt/pypackages"):
    if _p not in sys.path and os.path.isdir(_p):
        sys.path.append(_p)

import ml_dtypes  # noqa: E402

DIM, BOX, H = 384, 4, 12
N = BOX ** 3            # 64 tokens per window
HD = DIM // H           # 32
SCALE = HD ** -0.5
B_ = 2048
NCORES = 8
B_PER = B_ // NCORES    # 256 windows per core
TOK = B_PER * N         # 16384 tokens per core
SUPER = 512             # tokens per super-tile (8 windows)
PAIR = 128              # tokens per pair-tile (2 windows)
UNROLL = 4              # super-tiles per hardware-loop iteration

MODE = os.environ.get("BOXATTN_MODE", "bf16")  # "f32" | "bf16"

_cache = {}


CFG = {
    "xt_bufs": 4, "qk_bufs": 3, "v_bufs": 4, "er_bufs": 3, "et_bufs": 3,
    "av_bufs": 3, "avt_bufs": 3, "o_bufs": 3, "inv_bufs": 3,
    "psA_bufs": 2, "psS_bufs": 4, "psB_bufs": 2,
    "skew": True, "dve_cols": 1024,
    "xt_dma": "sync", "avt_dma": "sync", "out_dma": "sync",
    "staggered": False, "hint_pe": True, "unroll": None,
    "addbias": False,
}


def _build(mode, tok_per_core, reps=1, static=False):
    import concourse.bass as bass
    import concourse.mybir as mybir
    import concourse.tile as tile
    from concourse import bacc

    f32 = mybir.dt.float32
    dt = mybir.dt.bfloat16 if mode == "bf16" else f32

    nc = bacc.Bacc("TRN2", target_bir_lowering=False, debug=False)

    x_d = nc.dram_tensor("x", [tok_per_core, DIM], dt, kind="ExternalInput").ap()
    wqk_d = nc.dram_tensor("wqkT", [DIM, 768], dt, kind="ExternalInput").ap()
    wv_d = nc.dram_tensor("wvT", [DIM, DIM], dt, kind="ExternalInput").ap()
    wp_d = nc.dram_tensor("wpT", [DIM, DIM], dt, kind="ExternalInput").ap()
    eb_d = nc.dram_tensor("ebT", [PAIR, 1536], dt, kind="ExternalInput").ap()
    pb_d = nc.dram_tensor("pb", [PAIR, DIM], f32, kind="ExternalInput").ap()
    id_d = nc.dram_tensor("ident", [PAIR, PAIR], f32, kind="ExternalInput").ap()
    out_d = nc.dram_tensor("out", [tok_per_core, DIM], f32, kind="ExternalOutput").ap()

    n_super = tok_per_core // SUPER
    unroll = CFG["unroll"] or UNROLL
    if n_super % unroll != 0:
        unroll = 1
    n_iter = n_super // unroll

    with tile.TileContext(nc) as tc:
        with (
            tc.tile_pool(name="consts", bufs=1) as consts,
            tc.tile_pool(name="xn", bufs=3) as xn_pool,
            tc.tile_pool(name="xt", bufs=CFG["xt_bufs"]) as xt_pool,
            tc.tile_pool(name="qk", bufs=CFG["qk_bufs"]) as qk_pool,
            tc.tile_pool(name="v", bufs=CFG["v_bufs"]) as v_pool,
            tc.tile_pool(name="er", bufs=CFG["er_bufs"]) as er_pool,
            tc.tile_pool(name="et", bufs=CFG["et_bufs"]) as et_pool,
            tc.tile_pool(name="av", bufs=CFG["av_bufs"]) as av_pool,
            tc.tile_pool(name="avt", bufs=CFG["avt_bufs"]) as avt_pool,
            tc.tile_pool(name="osb", bufs=CFG["o_bufs"]) as o_pool,
            tc.tile_pool(name="inv", bufs=CFG["inv_bufs"]) as inv_pool,
            tc.tile_pool(name="psA", bufs=CFG["psA_bufs"], space="PSUM") as psA,
            tc.tile_pool(name="psS", bufs=CFG["psS_bufs"], space="PSUM") as psS,
            tc.tile_pool(name="psB", bufs=CFG["psB_bufs"], space="PSUM") as psB,
        ):
            wqk = consts.tile([128, 3, 768], dt)
            nc.sync.dma_start(wqk[:], wqk_d.rearrange("(a p) k -> p a k", p=128))
            wv = consts.tile([128, 3, DIM], dt)
            nc.sync.dma_start(wv[:], wv_d.rearrange("(a p) k -> p a k", p=128))
            wp = consts.tile([128, 3, DIM], dt)
            nc.sync.dma_start(wp[:], wp_d.rearrange("(a p) k -> p a k", p=128))
            eb = consts.tile([PAIR, 1536], dt)
            nc.sync.dma_start(eb[:], eb_d)
            pb = consts.tile([PAIR, DIM], f32)
            nc.sync.dma_start(pb[:], pb_d)
            ident = None
            if mode != "bf16":
                ident = consts.tile([PAIR, PAIR], f32)
                nc.sync.dma_start(ident[:], id_d)
            identb = None
            if CFG["addbias"]:
                identf = consts.tile([PAIR, PAIR], f32, tag="identf")
                nc.sync.dma_start(identf[:], id_d)
                identb = consts.tile([PAIR, PAIR], dt, tag="identb")
                nc.vector.tensor_copy(identb[:], identf[:])

            def stage_head(t0):
                """xT load + q,k projections for one super-tile.
                Returns (xt, qkt) tiles."""
                xt = xt_pool.tile([128, 3, SUPER], dt, tag="xt")
                if mode == "bf16":
                    if CFG.get("fused_xt"):
                        getattr(nc, CFG["xt_dma"]).dma_start(
                            out=xt[:],
                            in_=x_d[bass.ds(t0, SUPER), :],
                            transpose=True,
                        )
                    else:
                        for cc in range(3):
                            getattr(nc, CFG["xt_dma"]).dma_start(
                                out=xt[:, cc, :],
                                in_=x_d[bass.ds(t0, SUPER), cc * 128 : (cc + 1) * 128],
                                transpose=True,
                            )
                else:
                    xn = xn_pool.tile([128, 4, DIM], f32, tag="xn")
                    nc.sync.dma_start(
                        xn[:],
                        x_d[bass.ds(t0, SUPER), :].rearrange("(b p) c -> p b c", p=128),
                    )
                    for cc in range(3):
                        for tb in range(4):
                            tp = psB.tile([128, 128], f32, tag="bp")
                            nc.tensor.transpose(
                                tp[:], xn[:, tb, cc * 128 : (cc + 1) * 128], ident[:]
                            )
                            nc.scalar.copy(xt[:, cc, tb * 128 : (tb + 1) * 128], tp[:])

                qkt = qk_pool.tile([128, 6, SUPER], dt, tag="qkt")
                for j in range(6):
                    ps = psA.tile([128, SUPER], f32, tag="psA")
                    for cc in range(3):
                        nc.tensor.matmul(
                            ps[:],
                            lhsT=wqk[:, cc, j * 128 : (j + 1) * 128],
                            rhs=xt[:, cc, :],
                            start=(cc == 0),
                            stop=(cc == 2),
                        )
                    # split psum->sbuf copies between ACT and DVE
                    if j % 2 == 0:
                        nc.scalar.copy(qkt[:, j, :], ps[:])
                    else:
                        nc.vector.tensor_copy(qkt[:, j, :], ps[:])
                return xt, qkt

            def stage_vS(st, blk):
                """v projection + S^T matmuls for pair `blk`."""
                xt, qkt = st["xt"], st["qkt"]
                vps = psA.tile([128, DIM], f32, tag="psA")
                for cc in range(3):
                    nc.tensor.matmul(
                        vps[:],
                        lhsT=xt[:, cc, blk * 128 : (blk + 1) * 128],
                        rhs=wv[:, cc, :],
                        start=(cc == 0),
                        stop=(cc == 2),
                    )
                vsb = v_pool.tile([128, H, 33], dt, tag="v")
                nc.vector.memset(vsb[:, :, 32:33], 1.0)
                nc.vector.tensor_copy(
                    vsb[:, :, 0:32], vps[:].rearrange("p (h d) -> p h d", d=32)
                )
                st["vsb"][blk] = vsb

                # S^T per head: cross-window blocks are garbage; the bias
                # multiply (eb = 0 there) zeroes them, making E^T
                # block-diagonal so AV is one matmul per head too.
                # One PSUM bank per PE row-group g=h%4 (concurrent
                # tile_position matmuls must not share a bank).
                sts = []
                for _g in range(4):
                    st_g = psS.tile([128, 384], f32, tag="s")
                    sts.append(st_g)
                addb = CFG["addbias"]
                if addb:
                    # bias lands first (start=True fills the whole bank);
                    # S matmuls then accumulate on top. Cross-window blocks
                    # hold -30000 so exp() zeroes them.
                    for g in range(4):
                        nc.tensor.matmul(
                            sts[g][:],
                            lhsT=identb[:],
                            rhs=eb[:, g * 384 : (g + 1) * 384],
                            start=True,
                            stop=False,
                        )
                for h in range(H):
                    g, j = h % 4, h // 4
                    rp = g * 32
                    f0 = blk * 128
                    nc.tensor.matmul(
                        sts[g][:, j * 128 : (j + 1) * 128],
                        lhsT=qkt[rp : rp + 32, 3 + j, f0 : f0 + 128],
                        rhs=qkt[rp : rp + 32, j, f0 : f0 + 128],
                        start=not addb,
                        stop=True,
                        tile_position=(rp, 0),
                    )
                st["sts"][blk] = sts

            def stage_exp(st, blk):
                """exp(S^T) (+ bias multiply unless folded into S) for `blk`."""
                sts = st["sts"][blk]
                et = et_pool.tile([128, 1536], dt, tag="et")
                if CFG["addbias"]:
                    for g in range(4):
                        nc.scalar.activation(
                            et[:, g * 384 : (g + 1) * 384],
                            sts[g][:],
                            mybir.ActivationFunctionType.Exp,
                        )
                else:
                    er = er_pool.tile([128, 1536], dt, tag="er")
                    for g in range(4):
                        nc.scalar.activation(
                            er[:, g * 384 : (g + 1) * 384],
                            sts[g][:],
                            mybir.ActivationFunctionType.Exp,
                        )
                    dc = CFG["dve_cols"]
                    nc.vector.tensor_mul(et[:, 0:dc], er[:, 0:dc], eb[:, 0:dc])
                    nc.gpsimd.tensor_mul(
                        et[:, dc:1536], er[:, dc:1536], eb[:, dc:1536]
                    )
                st["et"][blk] = et

            def stage_av(st, blk):
                """AV matmuls + softmax normalize + attn^T for pair `blk`."""
                et, vsb = st["et"][blk], st["vsb"][blk]
                avp_t = psB.tile([128, 512], f32, tag="bp")
                avp = avp_t[:, 0 : H * 33].rearrange("p (h d) -> p h d", d=33)
                for h in range(H):
                    ec = (h % 4) * 384 + (h // 4) * 128
                    nc.tensor.matmul(
                        avp[:, h, :],
                        lhsT=et[:, ec : ec + 128],
                        rhs=vsb[:, h, :],
                        start=True,
                        stop=True,
                    )
                inv = inv_pool.tile([128, H], f32, tag="inv")
                nc.vector.reciprocal(inv[:], avp[:, :, 32])
                avsb = av_pool.tile([128, H, 32], dt, tag="av")
                nc.vector.tensor_mul(
                    avsb[:],
                    avp[:, :, 0:32],
                    inv[:, :, None].broadcast_to([128, H, 32]),
                )
                avt = avt_pool.tile([128, 3, 128], dt, tag="avt")
                if mode == "bf16":
                    if CFG.get("avt_split"):
                        flat = avsb[:].rearrange("p h d -> p (h d)")
                        for cc in range(3):
                            getattr(nc, CFG["avt_dma"]).dma_start(
                                out=avt[:, cc, :],
                                in_=flat[:, cc * 128 : (cc + 1) * 128],
                                transpose=True,
                            )
                    else:
                        getattr(nc, CFG["avt_dma"]).dma_start(
                            out=avt[:],
                            in_=avsb[:].rearrange("p h d -> p (h d)"),
                            transpose=True,
                        )
                else:
                    for cc in range(3):
                        tp = psB.tile([128, 128], f32, tag="bp")
                        nc.tensor.transpose(
                            tp[:],
                            avsb[:].rearrange("p h d -> p (h d)")[
                                :, cc * 128 : (cc + 1) * 128
                            ],
                            ident[:],
                        )
                        nc.scalar.copy(avt[:, cc, :], tp[:])
                st["avt"][blk] = avt

            def stage_out(st, t0, blk):
                """output projection + bias + store for pair `blk`."""
                avt = st["avt"][blk]
                ops = psA.tile([128, DIM], f32, tag="psA")
                for cc in range(3):
                    nc.tensor.matmul(
                        ops[:],
                        lhsT=avt[:, cc, :],
                        rhs=wp[:, cc, :],
                        start=(cc == 0),
                        stop=(cc == 2),
                    )
                osb = o_pool.tile([128, DIM], f32, tag="osb")
                nc.vector.tensor_add(osb[:], ops[:], pb[:])
                getattr(nc, CFG["out_dma"]).dma_start(
                    out_d[bass.ds(t0 + blk * PAIR, PAIR), :], osb[:]
                )

            def emit_supers(t0s):
                """Emit a software-pipelined run over the pair-jobs of the
                given super-tile offsets. Later stages of earlier pairs are
                emitted BEFORE each pair's v+S, so the in-order PE stream
                always has independent matmuls between dependent stages."""
                if not CFG["skew"]:
                    for t0 in t0s:
                        xt, qkt = stage_head(t0)
                        st = {"xt": xt, "qkt": qkt, "vsb": {}, "sts": {},
                              "et": {}, "avt": {}}
                        for blk in range(4):
                            stage_vS(st, blk)
                            stage_exp(st, blk)
                            stage_av(st, blk)
                            stage_out(st, t0, blk)
                    return
                jobs = [(si, blk) for si in range(len(t0s)) for blk in range(4)]
                sts_ = {}
                n = len(jobs)
                osk = CFG.get("out_skew", 3)
                head_last = CFG.get("head_last", False)
                for i in range(n + osk):
                    if not head_last and i < n and jobs[i][1] == 0:
                        si = jobs[i][0]
                        xt, qkt = stage_head(t0s[si])
                        sts_[si] = {"xt": xt, "qkt": qkt, "vsb": {}, "sts": {},
                                    "et": {}, "avt": {}}
                    if 1 <= i <= n:
                        si, blk = jobs[i - 1]
                        stage_exp(sts_[si], blk)
                    if 2 <= i <= n + 1:
                        si, blk = jobs[i - 2]
                        stage_av(sts_[si], blk)
                    if osk <= i <= n + osk - 1:
                        si, blk = jobs[i - osk]
                        stage_out(sts_[si], t0s[si], blk)
                    if head_last and i < n and jobs[i][1] == 0:
                        si = jobs[i][0]
                        xt, qkt = stage_head(t0s[si])
                        sts_[si] = {"xt": xt, "qkt": qkt, "vsb": {}, "sts": {},
                                    "et": {}, "avt": {}}
                    if i < n:
                        si, blk = jobs[i]
                        stage_vS(sts_[si], blk)

            if static:
                for it in range(reps * n_iter):
                    base = (it % n_iter) * (SUPER * unroll)
                    emit_supers([base + u * SUPER for u in range(unroll)])
            else:
                with tc.For_i(0, reps) as _rep:
                    with tc.For_i(
                        0,
                        n_iter,
                        hint_engines=(
                            (mybir.EngineType.PE,) if CFG["hint_pe"] else ()
                        ),
                        staggered_reset=CFG["staggered"],
                    ) as sp:
                        base = sp * (SUPER * unroll)
                        emit_supers([base + u * SUPER for u in range(unroll)])
    nc.compile()
    return nc


def _get_nc(mode, tok_per_core, reps=1):
    key = (mode, tok_per_core, reps)
    if key not in _cache:
        _cache[key] = _build(mode, tok_per_core, reps)
    return _cache[key]


def _host_prep(x, qkv_w, proj_w, proj_b, bias_table, rel_idx, mode, n_cores):
    np_dt = ml_dtypes.bfloat16 if mode == "bf16" else np.float32
    x = np.asarray(x, np.float32)
    qkv_w = np.asarray(qkv_w, np.float32)
    proj_w = np.asarray(proj_w, np.float32)
    proj_b = np.asarray(proj_b, np.float32)
    bias_table = np.asarray(bias_table, np.float32)
    rel_idx = np.asarray(rel_idx)

    wq = qkv_w[0:DIM] * SCALE
    wk = qkv_w[DIM : 2 * DIM]
    wv = qkv_w[2 * DIM :]
    wqkT = np.concatenate([wq, wk], 0).T.copy().astype(np_dt)  # [384, 768]
    wvT = wv.T.copy().astype(np_dt)
    wpT = proj_w.T.copy().astype(np_dt)

    bias = bias_table[rel_idx.reshape(-1)].reshape(N, N, H)  # [nq, nk, h]
    if CFG["addbias"]:
        # additive form: S += bias; cross-window blocks get -30000 so
        # exp() zeroes them (fp32 psum keeps the -30000 dominant).
        eb1 = bias.transpose(1, 2, 0)  # [nk, h, nq]
        ebT = np.full((PAIR, H * PAIR), -30000.0, np.float32)
    else:
        eb1 = np.exp(bias).transpose(1, 2, 0)  # [nk, h, nq]
        ebT = np.zeros((PAIR, H * PAIR), np.float32)  # cross blocks stay 0
    for h in range(H):
        ec = (h % 4) * 384 + (h // 4) * 128
        for w in range(2):
            ebT[w * N : (w + 1) * N, ec + w * N : ec + (w + 1) * N] = eb1[:, h, :]
    ebT = ebT.astype(np_dt)  # [128, 1536]
    pb = np.broadcast_to(proj_b, (PAIR, DIM)).copy().astype(np.float32)
    ident = np.eye(PAIR, dtype=np.float32)

    B = x.shape[0]
    bper = B // n_cores
    xs = x.reshape(B * N, DIM).astype(np_dt)
    in_maps = []
    for c in range(n_cores):
        in_maps.append(
            {
                "x": xs[c * bper * N : (c + 1) * bper * N],
                "wqkT": wqkT,
                "wvT": wvT,
                "wpT": wpT,
                "ebT": ebT,
                "pb": pb,
                "ident": ident,
            }
        )
    return in_maps


def kernel(x, qkv_w, proj_w, proj_b, bias_table, rel_idx):
    import time

    from concourse.bass_utils import run_bass_kernel_spmd

    x = np.asarray(x)
    B = x.shape[0]
    n_cores = NCORES
    tok_per_core = (B // n_cores) * N
    nc = _get_nc(MODE, tok_per_core)
    in_maps = _host_prep(x, qkv_w, proj_w, proj_b, bias_table, rel_idx, MODE, n_cores)
    try:
        res = run_bass_kernel_spmd(nc, in_maps, list(range(n_cores)))
    except Exception:
        # transient device wedge (e.g. NRT_EXEC_UNIT_UNRECOVERABLE) — retry
        time.sleep(5)
        res = run_bass_kernel_spmd(nc, in_maps, list(range(n_cores)))
    out = np.concatenate([r["out"] for r in res.results], 0)
    return out.reshape(B, N, DIM).astype(np.float32)
